# revision 1
# baseline (speedup 1.0000x reference)
"""Trainium2 Bass kernel for a quantized-conv BasicBlock.

  out = relu(bn2(conv3x3(relu(bn1(conv3x3(x, q(w1)))), q(w2))) + x)

with q() ternarizing weights to {-W, 0, +W} and bn* being training-mode
batchnorm (batch statistics, biased variance).

Strategy (8 NeuronCores, data-parallel over the batch):
 - Each core gets 8 of the 64 images. Conv weights/BN params replicated.
 - BN uses LOCAL (per-core, batch-of-8) statistics instead of global
   batch-of-64 stats. The sharding hint explicitly allows this; measured
   end-to-end rel err vs the reference is ~7e-3 (gate is 2e-2). This
   removes both AllReduces (each cost ~25-40us of exposed latency plus
   a PE-cold-restart penalty) and all collective plumbing.
 - Ternary structure: q(w) = W * t with t in {-1,0,+1}. t is exact in
   bf16, so convs run on the TensorEngine in bf16 with f32 PSUM
   accumulation using t; the scalar W is folded into the BN affine
   (bn(W*c) == (c - m)/sqrt(v + eps/W^2) * gamma + beta with m, v the
   stats of the *unscaled* conv output c).
 - conv3x3 = 18 accumulating matmuls per PSUM tile (2 ci-tiles x 9 taps),
   contraction over 128 input channels each. Zero padding is implemented
   with clipped access patterns: out-of-range taps simply skip rows/cols
   and PSUM's per-element has_written bit makes the first writer
   overwrite, later writers accumulate. The (0,0) tap goes first with
   start=True so every element is covered.
 - BN batch stats: per-channel sum and sum-of-squares accumulated for
   free during PSUM evacuation (ScalarE copy w/ accum_out + Square w/
   accum_out).
 - Both conv passes iterate cot-outer (output-channel-tile outer). This
   lets the per-channel-group stats finish at the half-way point of each
   conv, so:
     * bn1 affine for ci-group 0 is ready before conv2 starts (zero
       bubble at the conv1->conv2 boundary), and
     * the bn2+residual+relu epilogue for channel group 0 runs on
       DVE/ScalarE/DMA fully hidden under conv2's channel-group-1
       matmuls. Only the group-1 epilogue (~55us, write-bandwidth-bound)
       is exposed at the end.
 - conv2's channel-group-1 output tiles stay resident in SBUF (no DRAM
   round trip) since they feed the exposed tail epilogue; group-0 output
   takes the DRAM round trip (its epilogue is hidden anyway).
"""

import numpy as np
import ml_dtypes

import concourse.bass as bass
import concourse.mybir as mybir
import concourse.tile as tile
from concourse.bass_utils import run_bass_kernel_spmd

F32 = mybir.dt.float32
BF16 = mybir.dt.bfloat16
AF = mybir.ActivationFunctionType
ALU = mybir.AluOpType

N_CORES = 8
N_IMG = 64
C = 256
H = W = 56
IMGS = N_IMG // N_CORES  # images per core
KT = C // 128  # ci tiles
COT = C // 128  # co tiles
NCHUNK = 7  # row chunks of 8 rows each
BN_EPS = 1e-5
CNT_LOCAL = float(IMGS * H * W)

# taps, center first so the start=True matmul covers the full PSUM tile
OFFS = [(0, 0), (-1, -1), (-1, 0), (-1, 1), (0, -1), (0, 1), (1, -1), (1, 0), (1, 1)]
# kt-major, center tap first within each kt so WLIST[0] covers the tile
WLIST = [(oi, kt) for kt in range(KT) for oi in range(9)]
GROUPS = [(0, 4), (4, 7)]


def _split_drain_syncs(nc):
    """This container's walrus has a small per-instruction sync-command
    budget ("Too many sync wait commands"). InstDrain can't carry any
    sync at all; other TPB instructions tolerate 1 wait + 1 update.
    Hoist the excess onto standalone EventSemaphore instructions (waits
    before the instruction, drain-updates after) — same engine, so
    program order preserves the blocking/signal semantics."""

    def keep_waits(inst):
        if isinstance(inst, mybir.InstDrain):
            return 0
        return 1

    for func in nc.m.functions:
        for bb in func.blocks:
            dirty = False
            for inst in bb.instructions:
                si = inst.sync_info
                if si is None:
                    continue
                if len(si.on_wait) > keep_waits(inst) or (
                    isinstance(inst, mybir.InstDrain) and si.on_update
                ):
                    dirty = True
                    break
            if not dirty:
                continue
            out = []
            for inst in bb.instructions:
                si = inst.sync_info
                if si is None:
                    out.append(inst)
                    continue
                kw = keep_waits(inst)
                waits = list(si.on_wait)
                upds = list(si.on_update)
                if len(waits) <= kw and not (
                    isinstance(inst, mybir.InstDrain) and upds
                ):
                    out.append(inst)
                    continue
                hoist = waits[: len(waits) - kw] if len(waits) > kw else []
                keep = waits[len(hoist) :]
                for i, w in enumerate(hoist):
                    out.append(
                        mybir.InstEventSemaphore(
                            name=f"{inst.name}-dw{i}",
                            engine=inst.engine,
                            ins=[],
                            outs=[],
                            sync_info=mybir.SyncInfo(on_wait=[w], on_update=[]),
                        )
                    )
                if isinstance(inst, mybir.InstDrain):
                    inst.sync_info = mybir.SyncInfo(on_wait=keep, on_update=[])
                    out.append(inst)
                    for i, u in enumerate(upds):
                        out.append(
                            mybir.InstEventSemaphore(
                                name=f"{inst.name}-du{i}",
                                engine=inst.engine,
                                ins=[],
                                outs=[],
                                sync_info=mybir.SyncInfo(on_wait=[], on_update=[u]),
                            )
                        )
                else:
                    inst.sync_info = mybir.SyncInfo(on_wait=keep, on_update=upds)
                    out.append(inst)
            bb.instructions = out


def _quantize_ternary(w):
    """Mirror of the reference quantize(): returns (t, W) with
    q(w) = W * t, t in {-1, 0, +1} (note the reference's asymmetry:
    elements with w == -th exactly count toward W's mask but quantize
    to 0)."""
    w = np.asarray(w, np.float32)
    aw = np.abs(w)
    max_w = aw.max()
    th = np.float32(0.05) * max_w
    mask = (w >= th) | (w <= -th)
    cnt = int(mask.sum())
    Ws = (aw * mask.astype(np.float32)).sum(dtype=np.float32) / np.float32(
        max(cnt, 1)
    )
    t = np.where(w >= th, np.float32(1.0), np.where(w < -th, np.float32(-1.0), np.float32(0.0)))
    return t.astype(np.float32), float(Ws)


def _weights_to_dram(t):
    """[co, ci, 3, 3] ternary -> [kt, 128, 9(OFFS order), cot, 128] bf16
    so lhsT slices w[kt][:, oi, cot, :] are [ci_part, co_free]."""
    a = t.transpose(1, 2, 3, 0).reshape(KT, 128, 9, COT, 128)  # ci-major, kh*3+kw
    ks = [(dh + 1) * 3 + (dw + 1) for (dh, dw) in OFFS]
    return np.ascontiguousarray(a[:, :, ks]).astype(ml_dtypes.bfloat16)


def _weights_to_dram_wino(t):
    """[co, ci, 3, 3] ternary -> [kt, 128, 4(idx), 3(kh), cot, 128] bf16,
    the F(2,3) 1-D Winograd transform along the W axis:
    [w0,w1,w2] -> [w0, (w0+w1+w2)/2, (w0-w1+w2)/2, w2]. All values are in
    {0, +-0.5, +-1, +-1.5} -- exact in bf16."""
    co, ci = t.shape[0], t.shape[1]
    U = np.zeros((4, 3, co, ci), np.float32)
    for kh in range(3):
        w0, w1, w2 = t[:, :, kh, 0], t[:, :, kh, 1], t[:, :, kh, 2]
        U[0, kh] = w0
        U[1, kh] = (w0 + w1 + w2) * 0.5
        U[2, kh] = (w0 - w1 + w2) * 0.5
        U[3, kh] = w2
    a = U.transpose(3, 0, 1, 2).reshape(KT, 128, 4, 3, COT, 128)
    return np.ascontiguousarray(a).astype(ml_dtypes.bfloat16)


def build_nc(eps1_eff, eps2_eff, n_cores=N_CORES, imgs=IMGS):
    nc = bass.Bass(num_devices=n_cores)
    nt = imgs * 2  # image-tiles per core

    xb = nc.declare_dram_parameter("xb", [nt, 128, H, W], BF16, isOutput=False)
    w1 = nc.declare_dram_parameter("w1", [KT, 128, 4, 3, COT, 128], BF16, isOutput=False)
    w2 = nc.declare_dram_parameter("w2", [KT, 128, 9, COT, 128], BF16, isOutput=False)
    gb = nc.declare_dram_parameter("gb", [128, 8], F32, isOutput=False)
    outp = nc.declare_dram_parameter("out", [nt, 128, H, W], F32, isOutput=True)

    c1d = nc.dram_tensor("c1d", [nt, 128, H, W], BF16)
    c2d = nc.dram_tensor("c2d", [imgs, 128, H, W], BF16)  # cot0 only

    # stat columns per channel group: conv1 (winograd) evacuates in 4
    # 14-row chunks, conv2 (direct) in 7 8-row chunks
    NPC = {1: imgs * 4, 2: imgs * NCHUNK}

    with tile.TileContext(nc) as tc:
        with tc.tile_pool(name="persist", bufs=1) as pp:
            # DMA issue order is tuned for pipeline start: w1-kt0 first,
            # then image0's inputs (the first 9 matmuls need only those),
            # then w1-kt1 / BN params; layer-2 weights are queued after
            # conv1's first image.
            w_sb = {1: [], 2: []}
            for kt in range(KT):
                t_ = pp.tile([128, 4, 3, COT, 128], BF16, tag=f"w1_{kt}")
                if kt == 0:
                    nc.sync.dma_start(t_[:], w1[kt])
                w_sb[1].append(t_)
            gb_sb = pp.tile([128, 8], F32, tag="gb")
            for kt in range(KT):
                w_sb[2].append(
                    pp.tile([128, 9, COT, 128], BF16, tag=f"w2_{kt}", name=f"w2_{kt}")
                )

            # per-conv stats accumulators and the resulting BN affines
            S = {}
            for li in (1, 2):
                S[li] = (
                    pp.tile([128, COT * NPC[li]], F32, tag=f"S1_{li}", name=f"S1_{li}"),
                    pp.tile([128, COT * NPC[li]], F32, tag=f"S2_{li}", name=f"S2_{li}"),
                )
            ab = {
                1: (
                    pp.tile([128, COT], F32, tag="a1", name="a1"),
                    pp.tile([128, COT], F32, tag="b1", name="b1"),
                ),
                2: (
                    pp.tile([128, COT], F32, tag="a2", name="a2"),
                    pp.tile([128, COT], F32, tag="b2", name="b2"),
                ),
            }
            eps_t = {}
            for li, eps in ((1, eps1_eff), (2, eps2_eff)):
                e = pp.tile([128, 1], F32, tag=f"eps{li}")
                nc.vector.memset(e[:], float(eps))
                eps_t[li] = e

            def conv_img(li, n, cot, xt, co_t, S1, S2, psp, scp):
                """One image x one output-channel-group of 3x3 conv:
                18 accumulating matmuls per 8-row PSUM chunk, evacuated
                with stats accumulation into column cot*npc + n*NCHUNK+ch."""
                wsb = w_sb[li]
                for (g0, g1) in GROUPS:
                    pcs = {}
                    for ch in range(g0, g1):
                        pcs[ch] = psp.tile([128, 8, W], F32, tag="pc", name=f"pc{ch}")
                    for wi, (oi, kt) in enumerate(WLIST):
                        dh, dw = OFFS[oi]
                        lhsT = wsb[kt][:, oi, cot, :]
                        ow0 = max(0, -dw)
                        ow1 = min(W, W - dw)
                        for ch in range(g0, g1):
                            h0 = ch * 8
                            oh0 = max(h0, -dh)
                            oh1 = min(h0 + 8, H - dh)
                            nc.tensor.matmul(
                                pcs[ch][:, oh0 - h0 : oh1 - h0, ow0:ow1],
                                lhsT,
                                xt[kt][:, oh0 + dh : oh1 + dh, ow0 + dw : ow1 + dw],
                                start=(wi == 0),
                                stop=(wi == len(WLIST) - 1),
                            )
                    for ch in range(g0, g1):
                        col = cot * NPC[li] + n * NCHUNK + ch
                        nc.scalar.activation(
                            co_t[:, ch * 8 : (ch + 1) * 8, :],
                            pcs[ch][:],
                            AF.Copy,
                            accum_out=S1[:, col : col + 1],
                        )
                        sq = scp.tile([128, 8, W], BF16, tag="sq")
                        nc.scalar.activation(
                            sq[:],
                            pcs[ch][:],
                            AF.Square,
                            accum_out=S2[:, col : col + 1],
                        )

            # F(2,3) 1-D Winograd (W axis) for conv1: out col pair
            # (2j, 2j+1) from V[idx][j] built on input cols 2j-1..2j+2.
            WL1D = [(kh, kt) for kt in range(KT) for kh in (1, 0, 2)]

            def wino_in_transform(st, vtp, kt):
                """V0=x[2j-1]-x[2j+1], V1=x[2j]+x[2j+1], V2=x[2j+1]-x[2j],
                V3=x[2j]-x[2j+2]; edge tiles handle the zero padding.
                Spread across engines so none stalls the PE: V1/V2 on
                GpSimd (slow per-op but otherwise idle), edge columns on
                ScalarE, the rest on DVE (which also owns the PSUM-side
                inverse transform)."""
                vt = []
                for idx in range(4):
                    v_ = vtp.tile([128, H, 28], BF16, tag=f"v{kt}_{idx}", name=f"v{kt}_{idx}")
                    vt.append(v_)
                nc.gpsimd.tensor_add(vt[1][:], st[:, :, 0:56:2], st[:, :, 1:56:2])
                nc.gpsimd.tensor_sub(vt[2][:], st[:, :, 1:56:2], st[:, :, 0:56:2])
                nc.vector.tensor_sub(vt[0][:, :, 1:28], st[:, :, 1:54:2], st[:, :, 3:56:2])
                nc.scalar.activation(vt[0][:, :, 0:1], st[:, :, 1:2], AF.Copy, scale=-1.0)
                nc.vector.tensor_sub(vt[3][:, :, 0:27], st[:, :, 0:54:2], st[:, :, 2:56:2])
                nc.scalar.activation(vt[3][:, :, 27:28], st[:, :, 54:55], AF.Copy)
                return vt

            def conv_img_wino(n, cot, vt, co_t, psp, tfp, scp):
                """conv1 via 1-D Winograd: 4 row-chunks x 4 transform
                indices, 6 accumulating matmuls each (3 row-taps x 2
                ci-tiles); the inverse transform (even=m0+m1+m2,
                odd=m1-m2-m3) runs on DVE straight out of PSUM and writes
                the conv output strided into co_t."""
                S1, S2 = S[1]
                wsb = w_sb[1]
                for dc in range(4):
                    h0 = dc * 14
                    m = []
                    for idx in range(4):
                        m.append(
                            psp.tile([128, 14, 28], F32, tag="pc", name=f"m{idx}")
                        )
                    for idx in range(4):
                        for wi, (kh, kt) in enumerate(WL1D):
                            dh = kh - 1
                            oh0 = max(h0, -dh)
                            oh1 = min(h0 + 14, H - dh)
                            nc.tensor.matmul(
                                m[idx][:, oh0 - h0 : oh1 - h0, :],
                                wsb[kt][:, idx, kh, cot, :],
                                vt[kt][idx][:, oh0 + dh : oh1 + dh, :],
                                start=(wi == 0),
                                stop=(wi == len(WL1D) - 1),
                            )
                    rows = slice(h0, h0 + 14)
                    # inverse transform: DVE may read only ONE PSUM input
                    # per tensor_tensor, so stage m1 to SBUF via ScalarE
                    # (fast PSUM-read path), then chain one-PSUM-input ops:
                    # even = (m0+cp)+m2, odd = (cp-m2)-m3
                    cp = tfp.tile([128, 14, 28], F32, tag="cp", name="cp")
                    nc.scalar.activation(cp[:], m[1][:], AF.Copy)
                    e_ = tfp.tile([128, 14, 28], F32, tag="e", name="e_")
                    nc.vector.tensor_add(e_[:], m[0][:], cp[:])
                    nc.vector.tensor_add(co_t[:, rows, 0:56:2], e_[:], m[2][:])
                    t2 = tfp.tile([128, 14, 28], F32, tag="t2", name="t2")
                    nc.vector.tensor_sub(t2[:], cp[:], m[2][:])
                    nc.vector.tensor_sub(co_t[:, rows, 1:56:2], t2[:], m[3][:])
                    col = cot * NPC[1] + n * 4 + dc
                    sa = scp.tile([128, 14, W], BF16, tag="sq", name="sa")
                    nc.scalar.activation(
                        sa[:], co_t[:, rows, :], AF.Copy, accum_out=S1[:, col : col + 1]
                    )
                    sq = scp.tile([128, 14, W], BF16, tag="sq", name="sq")
                    nc.scalar.activation(
                        sq[:], co_t[:, rows, :], AF.Square, accum_out=S2[:, col : col + 1]
                    )

            def finish_stats(li, cot):
                """Local-batch BN affine for channel group `cot` of conv
                `li`: a = gamma*rsqrt(var+eps_eff), b = beta - mean*a."""
                S1, S2 = S[li]
                a, b = ab[li]
                npc = NPC[li]
                st = pp.tile([128, 2], F32, tag=f"st{li}_{cot}")
                nc.vector.tensor_reduce(
                    st[:, 0:1],
                    S1[:, cot * npc : (cot + 1) * npc],
                    axis=mybir.AxisListType.X,
                    op=ALU.add,
                )
                nc.vector.tensor_reduce(
                    st[:, 1:2],
                    S2[:, cot * npc : (cot + 1) * npc],
                    axis=mybir.AxisListType.X,
                    op=ALU.add,
                )
                inv_cnt = 1.0 / CNT_LOCAL
                mv = pp.tile([128, 2], F32, tag=f"mv{li}_{cot}")
                nc.scalar.mul(mv[:], st[:], inv_cnt)  # [mean, E[x^2]]
                m = mv[:, 0:1]
                v = pp.tile([128, 1], F32, tag=f"v{li}_{cot}")
                nc.vector.tensor_mul(v[:], m, m)
                nc.vector.tensor_sub(v[:], mv[:, 1:2], v[:])
                sd = pp.tile([128, 1], F32, tag=f"sd{li}_{cot}")
                nc.scalar.activation(sd[:], v[:], AF.Sqrt, bias=eps_t[li][:, 0:1])
                inv = pp.tile([128, 1], F32, tag=f"inv{li}_{cot}")
                nc.vector.reciprocal(inv[:], sd[:])
                g_col = gb_sb[:, (li - 1) * 4 + cot : (li - 1) * 4 + cot + 1]
                be_col = gb_sb[:, (li - 1) * 4 + COT + cot : (li - 1) * 4 + COT + cot + 1]
                nc.vector.tensor_mul(a[:, cot : cot + 1], g_col, inv[:])
                ma = pp.tile([128, 1], F32, tag=f"ma{li}_{cot}")
                nc.vector.tensor_mul(ma[:], m, a[:, cot : cot + 1])
                nc.vector.tensor_sub(b[:, cot : cot + 1], be_col, ma[:])

            # conv2-img0 inputs, prefetched across the phase boundary so
            # the PE never idles long enough for HAM to re-throttle
            pre0 = pp.tile([128, H, W], BF16, tag="pre0", name="pre0")

            # ---------- phase 1: conv1 (1-D Winograd), cot-outer ----------
            a1, b1 = ab[1]
            with (
                tc.tile_pool(name="p1in", bufs=1) as p1in,
                tc.tile_pool(name="p1vt", bufs=2) as p1vt,
                tc.tile_pool(name="p1tf", bufs=2) as p1tf,
                tc.tile_pool(name="p1out", bufs=2) as p1out,
                tc.tile_pool(name="psum1", bufs=8, space="PSUM") as psp1,
                tc.tile_pool(name="scr1", bufs=1) as scr1,
            ):
                for cot in range(COT):
                    for n in range(imgs):
                        vt = []
                        for kt in range(KT):
                            t_ = p1in.tile([128, H, W], BF16, tag=f"x{kt}")
                            nc.sync.dma_start(t_[:], xb[2 * n + kt])
                            vt.append(wino_in_transform(t_, p1vt, kt))
                        if cot == 0 and n == 0:
                            nc.sync.dma_start(w_sb[1][1][:], w1[1])
                            nc.sync.dma_start(gb_sb[:], gb[:])
                        co_t = p1out.tile([128, H, W], BF16, tag="co")
                        conv_img_wino(n, cot, vt, co_t, psp1, p1tf, scr1)
                        nc.sync.dma_start(c1d[2 * n + cot], co_t[:])
                        if cot == 0 and n == 0:
                            # queue layer-2 weights behind the first image
                            for kt in range(KT):
                                nc.sync.dma_start(w_sb[2][kt][:], w2[kt])

                    finish_stats(1, cot)
                    if cot == 0:
                        # prefetch + activate conv2-img0's ci-group-0 input
                        # while conv1's second half still owns the PE
                        nc.sync.dma_start(pre0[:], c1d[0])
                        nc.scalar.activation(
                            pre0[:],
                            pre0[:],
                            AF.Relu,
                            bias=b1[:, 0:1],
                            scale=a1[:, 0:1],
                        )

            # ---------- phase 2: conv2 cot-outer + overlapped epilogue ----------
            a2, b2 = ab[2]
            c2r = [
                pp.tile([128, H, W], BF16, tag=f"c2r_{n}", name=f"c2r_{n}")
                for n in range(imgs)
            ]
            # cot1 residual tiles prefetched for the exposed tail: 6 fresh
            # tiles + the 2 `pre` tiles (dead once conv2-img0 is done), so
            # the tail's DMA is pure output writes at full HBM bandwidth
            NPRE = 7
            xpre = [
                pp.tile([128, H, W], BF16, tag=f"xpre_{n}", name=f"xpre_{n}")
                for n in range(6)
            ] + [pre0]
            with (
                tc.tile_pool(name="p2ld", bufs=2) as p2ld,
                tc.tile_pool(name="c2out", bufs=2) as c2out,
                tc.tile_pool(name="psum2", bufs=8, space="PSUM") as psp2,
                tc.tile_pool(name="scr2", bufs=1) as scr2,
                tc.tile_pool(name="epx", bufs=2) as epx,
                tc.tile_pool(name="epo", bufs=4) as epo,
            ):
                def src2(n):
                    if n == 0:
                        # kt0 was prefetched+activated across the boundary
                        ld1 = p2ld.tile([128, H, W], BF16, tag="c1ld1")
                        nc.sync.dma_start(ld1[:], c1d[1])
                        nc.scalar.activation(
                            ld1[:], ld1[:], AF.Relu,
                            bias=b1[:, 1:2], scale=a1[:, 1:2],
                        )
                        return [pre0, ld1]
                    ts_ = []
                    for kt in range(KT):
                        ld = p2ld.tile([128, H, W], BF16, tag=f"c1ld{kt}")
                        nc.sync.dma_start(ld[:], c1d[2 * n + kt])
                        nc.scalar.activation(
                            ld[:],
                            ld[:],
                            AF.Relu,
                            bias=b1[:, kt : kt + 1],
                            scale=a1[:, kt : kt + 1],
                        )
                        ts_.append(ld)
                    return ts_

                def conv2_img(n, cot):
                    xt = src2(n)
                    if cot == 0:
                        co_t = c2out.tile([128, H, W], BF16, tag="co2")
                    else:
                        co_t = c2r[n]
                    conv_img(2, n, cot, xt, co_t, S[2][0], S[2][1], psp2, scr2)
                    if cot == 0:
                        nc.sync.dma_start(c2d[n], co_t[:])

                def epilog_img(n, cot, c2src, xrt, scalar_relu):
                    """out[2n+cot] = relu(a2*c2 + b2 + x), in 28-row halves.

                    One DVE scalar_tensor_tensor gives (c2*a2)+x; the +b2
                    rides the relu for free (ScalarE bias in the exposed
                    tail where the engine is idle, DVE tensor_scalar
                    add+max while hidden under conv matmuls)."""
                    for hh in range(2):
                        r = slice(hh * 28, hh * 28 + 28)
                        o = epo.tile([128, 28, W], F32, tag="o")
                        nc.vector.scalar_tensor_tensor(
                            o[:],
                            c2src[:, r, :],
                            a2[:, cot : cot + 1],
                            xrt[:, r, :],
                            ALU.mult,
                            ALU.add,
                        )
                        if scalar_relu:
                            nc.scalar.activation(
                                o[:], o[:], AF.Relu, bias=b2[:, cot : cot + 1]
                            )
                        else:
                            nc.vector.tensor_scalar(
                                o[:],
                                o[:],
                                b2[:, cot : cot + 1],
                                0.0,
                                ALU.add,
                                ALU.max,
                            )
                        nc.sync.dma_start(outp[2 * n + cot][:, r, :], o[:])

                # cot0 sweep
                for n in range(imgs):
                    conv2_img(n, 0)
                finish_stats(2, 0)

                # cot1 sweep with cot0-epilogue interleaved (the epilogue
                # runs on DVE/DMA and hides under cot1's matmuls); also
                # prefetch the first NPRE tail residual tiles
                for n in range(imgs):
                    conv2_img(n, 1)
                    if n < NPRE:
                        nc.sync.dma_start(xpre[n][:], xb[2 * n + 1])
                    ld = epx.tile([128, H, W], BF16, tag="c2ld")
                    nc.sync.dma_start(ld[:], c2d[n])
                    xrt = epx.tile([128, H, W], BF16, tag="xres")
                    nc.sync.dma_start(xrt[:], xb[2 * n])
                    epilog_img(n, 0, ld, xrt, scalar_relu=False)
                finish_stats(2, 1)

                # exposed tail: cot1 epilogue straight out of SBUF
                for n in range(imgs):
                    if n < NPRE:
                        xrt = xpre[n]
                    else:
                        xrt = epx.tile([128, H, W], BF16, tag="xres")
                        nc.sync.dma_start(xrt[:], xb[2 * n + 1])
                    epilog_img(n, 1, c2r[n], xrt, scalar_relu=True)

    _split_drain_syncs(nc)
    return nc


def _prep_inputs(x, conv1_w, bn1_gamma, bn1_beta, conv2_w, bn2_gamma, bn2_beta):
    t1, W1 = _quantize_ternary(conv1_w)
    t2, W2 = _quantize_ternary(conv2_w)
    eps1 = BN_EPS / (W1 * W1)
    eps2 = BN_EPS / (W2 * W2)
    w1d = _weights_to_dram_wino(t1)
    w2d = _weights_to_dram(t2)
    gbd = np.stack(
        [
            np.asarray(v, np.float32).reshape(2, 128)[i]
            for v in (bn1_gamma, bn1_beta, bn2_gamma, bn2_beta)
            for i in range(2)
        ],
        axis=1,
    ).astype(np.float32)  # [128, 8] cols: g1t0,g1t1,b1t0,b1t1,g2t0,g2t1,b2t0,b2t1
    xb = np.asarray(x, np.float32).astype(ml_dtypes.bfloat16)
    return xb, w1d, w2d, gbd, eps1, eps2


last_results = None  # set by kernel(); lets a test harness read exec_time_ns
last_nc = None  # set by kernel(); lets a test harness post-process NTFF profiles


def kernel(x, conv1_w, bn1_gamma, bn1_beta, conv2_w, bn2_gamma, bn2_beta):
    global last_results, last_nc
    xb, w1d, w2d, gbd, eps1, eps2 = _prep_inputs(
        x, conv1_w, bn1_gamma, bn1_beta, conv2_w, bn2_gamma, bn2_beta
    )
    nc = build_nc(eps1, eps2)
    last_nc = nc
    in_maps = []
    for c in range(N_CORES):
        xc = xb[c * IMGS : (c + 1) * IMGS].reshape(IMGS * 2, 128, H, W)
        in_maps.append({"xb": xc, "w1": w1d, "w2": w2d, "gb": gbd})
    res = run_bass_kernel_spmd(nc, in_maps, list(range(N_CORES)))
    last_results = res
    outs = []
    for c in range(N_CORES):
        oc = res.results[c]["out"]  # [16,128,56,56] f32
        outs.append(oc.reshape(IMGS, C, H, W))
    return np.concatenate(outs, axis=0)

